# revision 17
# baseline (speedup 1.0000x reference)
"""CIN (Compressed Interaction Network) kernel for Trainium2, 8 NeuronCores.

Math (per reference):
  layer l: cur[b,k,e] = sum_{i,j} x[b,i,e] * h[b,j,e] * Wl[i*fi+j, k] + bl[k]
  h_{l+1} = cur[:, :64, :] (layers 0,1); direct_l = cur[:, 64:, :] (l<2) / cur (l=2)
  out = sum_e concat(direct_0, direct_1, direct_2)  -> (B, 256)

Mapping: pure data-parallel over batch (128 rows per core). Per core, columns
n = (b_local, e) -> 8192. The bilinear contraction is done by materializing
Z[m, n] = x[i(m), n] * h[j(m), n] (m = i*fi+j over SBUF partitions, 128-row
chunks) with one DVE multiply per layer-tile (replicated x-operands are
prepared host-side and streamed from HBM), then PE matmuls W_chunk^T @ Z_chunk
accumulated in PSUM. Hidden halves get bias via ScalarE copy (stacked 2x for
the next layer's j-replication); direct halves are e-reduced on DVE. Direct
bias contributions (E * bl[k]) are added on host after gathering.

Everything heavy runs in fp16 operands with fp32 PSUM accumulation.
"""
import os
import sys

import numpy as np

for _p in ("/opt/trn_rl_repo", "/root/.axon_site/_ro/trn_rl_repo"):
    if os.path.isdir(_p) and _p not in sys.path:
        sys.path.append(_p)

import bass_rust
import concourse.bass as bass
import concourse.mybir as mybir
import concourse.tile as tile
from concourse.bass_utils import run_bass_kernel_spmd

F16 = mybir.dt.float16
F32 = mybir.dt.float32

N_CORES = 8
B, F0, E = 1024, 32, 64
BSH = B // N_CORES          # 128 batch rows per core
BE = BSH * E                # 8192 columns per core
NT = 512                    # tile width (8 b-rows x 64 e)
T = BE // NT                # 16 tiles
NB = NT // E                # b-rows per tile (8)

_CACHE = {}
last_exec_time_ns = None
last_results = None


def _split_multiwaits(nc):
    """The walrus build here fits only ONE sync-wait per engine instruction.
    Tile emits several; split the extras onto NoOps inserted just before."""
    ctr = 0
    for f in nc.m.functions:
        for bb in f.blocks:
            insts = bb.instructions
            k = 0
            while k < len(insts):
                i = insts[k]
                si = i.sync_info
                if si is not None and si.on_wait and len(si.on_wait) > 1:
                    waits = list(si.on_wait)
                    for w in waits[:-1]:
                        nop = bass_rust.InstNoOp(name=f"I-wsplit-{ctr}")
                        ctr += 1
                        nop.engine = i.engine
                        nop.sync_info = mybir.SyncInfo(on_wait=[w], on_update=[])
                        nc.register_instruction(nop)
                        insts.insert(k, nop)
                        k += 1
                    i.sync_info = mybir.SyncInfo(
                        on_wait=[waits[-1]], on_update=list(si.on_update or []))
                k += 1
    return nc


def _build_module(repeat=1):
    nc = bass.Bass(target_bir_lowering=False)

    a0_d = nc.dram_tensor("a0", [8, 128, BE], F16, kind="ExternalInput")
    a1_d = nc.dram_tensor("a1", [16, 128, BE], F16, kind="ExternalInput")
    x4_d = nc.dram_tensor("x4", [128, BE], F16, kind="ExternalInput")
    w0_d = nc.dram_tensor("w0", [8, 128, 128], F16, kind="ExternalInput")
    w1_d = nc.dram_tensor("w1", [16, 128, 128], F16, kind="ExternalInput")
    w2_d = nc.dram_tensor("w2", [16, 128, 128], F16, kind="ExternalInput")
    b0_d = nc.dram_tensor("b0h", [64, 1], F32, kind="ExternalInput")
    b1_d = nc.dram_tensor("b1h", [64, 1], F32, kind="ExternalInput")
    id_d = nc.dram_tensor("ident", [128, 128], F32, kind="ExternalInput")
    out_d = nc.dram_tensor("out", [128, 256], F32, kind="ExternalOutput")

    with tile.TileContext(nc) as tc:
        with (
            tc.tile_pool(name="const", bufs=1) as cpool,
            tc.tile_pool(name="a0p", bufs=2) as a0p,
            tc.tile_pool(name="a1p", bufs=2) as a1p,
            tc.tile_pool(name="z1p", bufs=2) as z1p,
            tc.tile_pool(name="h2p", bufs=2) as h2p,
            tc.tile_pool(name="psum", bufs=3, space="PSUM") as pspool,
            tc.tile_pool(name="pst", bufs=1, space="PSUM") as tppool,
        ):
            # --- constants / weights ---
            X4 = cpool.tile([128, BE], F16)
            nc.sync.dma_start(X4[:], x4_d[:])
            W0 = cpool.tile([128, 8, 128], F16)
            nc.sync.dma_start(W0[:], w0_d[:].transpose([1, 0, 2]))
            W1 = cpool.tile([128, 16, 128], F16)
            nc.sync.dma_start(W1[:], w1_d[:].transpose([1, 0, 2]))
            W2 = cpool.tile([128, 16, 128], F16)
            nc.sync.dma_start(W2[:], w2_d[:].transpose([1, 0, 2]))
            B0 = cpool.tile([64, 1], F32)
            nc.sync.dma_start(B0[:], b0_d[:])
            B1 = cpool.tile([64, 1], F32)
            nc.sync.dma_start(B1[:], b1_d[:])
            IDT = cpool.tile([128, 128], F32)
            nc.sync.dma_start(IDT[:], id_d[:])

            S0 = cpool.tile([64, 128], F32)   # e-sums of direct part, layer 0
            S1 = cpool.tile([64, 128], F32)
            S2 = cpool.tile([128, 128], F32)

            # Tiny per-engine reads of the constant tiles so loop instructions
            # never need a second sync-wait slot (engine instruction encodings
            # fit only one wait; these absorb the const-DMA deps into each
            # engine's vector clock).
            scratch = cpool.tile([1, 2], F16)
            nc.vector.tensor_copy(scratch[:], X4[0:1, 0:2])
            scratch2 = cpool.tile([64, 2], F32)
            nc.scalar.activation(scratch2[0:1, 0:1], B0[0:1, 0:1],
                                 mybir.ActivationFunctionType.Identity,
                                 bias=B0[0:1, :])
            nc.scalar.activation(scratch2[0:1, 1:2], B1[0:1, 0:1],
                                 mybir.ActivationFunctionType.Identity,
                                 bias=B1[0:1, :])
            pwarm = tppool.tile([1, 4], F32, tag="warm")
            nc.tensor.matmul(pwarm[0:1, 0:1], W0[0:1, 0:1, 0:1], W0[0:1, 0:1, 0:1])
            nc.tensor.matmul(pwarm[0:1, 1:2], W1[0:1, 0:1, 0:1], W1[0:1, 0:1, 0:1])
            nc.tensor.matmul(pwarm[0:1, 2:3], W2[0:1, 0:1, 0:1], W2[0:1, 0:1, 0:1])
            nc.tensor.matmul(pwarm[0:1, 3:4], IDT[0:1, 0:1], IDT[0:1, 0:1])

            def tile_body(t):
                cs = slice(t * NT, (t + 1) * NT)

                # ---- layer 0: Z0 = A0 * x4 (in place), cur0 = sum_c W0c^T Z0c
                A0t = a0p.tile([128, 8, NT], F16)
                nc.sync.dma_start(A0t[:], a0_d[:, :, cs].transpose([1, 0, 2]))
                in1 = X4[:, cs].unsqueeze(1).broadcast_to((128, 8, NT))
                nc.vector.tensor_mul(A0t[:], A0t[:], in1)
                cur0 = pspool.tile([128, NT], F32, tag="cur")
                for c in range(8):
                    nc.tensor.matmul(cur0[:], W0[:, c, :], A0t[:, c, :],
                                     start=(c == 0), stop=(c == 7))
                # hidden h1 (+bias), stacked 2x for j-replication
                H2a = h2p.tile([128, NT], F16, tag="h2a")
                nc.scalar.activation(H2a[0:64, :], cur0[0:64, :],
                                     mybir.ActivationFunctionType.Identity,
                                     bias=B0[:])
                nc.scalar.activation(H2a[64:128, :], cur0[0:64, :],
                                     mybir.ActivationFunctionType.Identity,
                                     bias=B0[:])
                # direct part e-sums
                nc.vector.reduce_sum(
                    S0[:, t * NB:(t + 1) * NB],
                    cur0[64:128, :].rearrange("p (s e) -> p s e", e=E),
                    axis=mybir.AxisListType.X)

                # ---- layer 1: Z1 = A1 * h1rep, cur1 = sum_c W1c^T Z1c
                A1t = a1p.tile([128, 16, NT], F16)
                nc.sync.dma_start(A1t[:], a1_d[:, :, cs].transpose([1, 0, 2]))
                Z1t = z1p.tile([128, 16, NT], F16)
                in1 = H2a[:].unsqueeze(1).broadcast_to((128, 16, NT))
                nc.vector.tensor_mul(Z1t[:], A1t[:], in1)
                cur1 = pspool.tile([128, NT], F32, tag="cur")
                for c in range(16):
                    nc.tensor.matmul(cur1[:], W1[:, c, :], Z1t[:, c, :],
                                     start=(c == 0), stop=(c == 15))
                H2b = h2p.tile([128, NT], F16, tag="h2b")
                nc.scalar.activation(H2b[0:64, :], cur1[0:64, :],
                                     mybir.ActivationFunctionType.Identity,
                                     bias=B1[:])
                nc.scalar.activation(H2b[64:128, :], cur1[0:64, :],
                                     mybir.ActivationFunctionType.Identity,
                                     bias=B1[:])
                nc.vector.reduce_sum(
                    S1[:, t * NB:(t + 1) * NB],
                    cur1[64:128, :].rearrange("p (s e) -> p s e", e=E),
                    axis=mybir.AxisListType.X)

                # ---- layer 2: Z2 = A1 * h2rep (in place), cur2 = sum W2c^T Z2c
                in1 = H2b[:].unsqueeze(1).broadcast_to((128, 16, NT))
                nc.vector.tensor_mul(A1t[:], A1t[:], in1)
                cur2 = pspool.tile([128, NT], F32, tag="cur")
                for c in range(16):
                    nc.tensor.matmul(cur2[:], W2[:, c, :], A1t[:, c, :],
                                     start=(c == 0), stop=(c == 15))
                nc.vector.reduce_sum(
                    S2[:, t * NB:(t + 1) * NB],
                    cur2[:].rearrange("p (s e) -> p s e", e=E),
                    axis=mybir.AxisListType.X)

            if repeat > 1:
                with tc.For_i(0, repeat, 1):
                    for t in range(T):
                        tile_body(t)
            else:
                for t in range(T):
                    tile_body(t)

            # ---- finalize: transpose (k, b) -> (b, k), assemble, store ----
            out_sb = cpool.tile([128, 256], F32)
            tp0 = tppool.tile([128, 64], F32, tag="tp")
            nc.tensor.transpose(tp0[:], S0[:], IDT[0:64, 0:64])
            nc.scalar.copy(out_sb[:, 0:64], tp0[:])
            tp1 = tppool.tile([128, 64], F32, tag="tp")
            nc.tensor.transpose(tp1[:], S1[:], IDT[0:64, 0:64])
            nc.scalar.copy(out_sb[:, 64:128], tp1[:])
            tp2 = tppool.tile([128, 128], F32, tag="tp2")
            nc.tensor.transpose(tp2[:], S2[:], IDT[:])
            nc.scalar.copy(out_sb[:, 128:256], tp2[:])
            nc.sync.dma_start(out_d[:], out_sb[:])

    return _split_multiwaits(nc)


def _get_runner(repeat=1):
    """Cached-jit executor mirroring bass2jax.run_bass_via_pjrt's multi-core
    path, so repeated calls skip re-tracing (for timing in test.py)."""
    key = ("runner", repeat)
    if key in _CACHE:
        return _CACHE[key]
    import jax
    from jax.sharding import Mesh, PartitionSpec
    from jax.experimental.shard_map import shard_map
    from concourse import bass2jax

    nc = _build_module(repeat)
    bass2jax.install_neuronx_cc_hook()
    partition_name = (nc.partition_id_tensor.name
                      if nc.partition_id_tensor else None)
    in_names, out_names, out_avals, zero_outs = [], [], [], []
    for alloc in nc.m.functions[0].allocations:
        if not isinstance(alloc, mybir.MemoryLocationSet):
            continue
        name = alloc.memorylocations[0].name
        if alloc.kind == "ExternalInput":
            if name != partition_name:
                in_names.append(name)
        elif alloc.kind == "ExternalOutput":
            shape = tuple(alloc.tensor_shape)
            dtype = mybir.dt.np(alloc.dtype)
            out_names.append(name)
            out_avals.append(jax.core.ShapedArray(shape, dtype))
            zero_outs.append(np.zeros(shape, dtype))
    n_params = len(in_names)
    all_in_names = list(in_names) + list(out_names)
    if partition_name is not None:
        all_in_names.append(partition_name)
    donate = tuple(range(n_params, n_params + len(out_names)))

    def _body(*args):
        operands = list(args)
        if partition_name is not None:
            operands.append(bass2jax.partition_id_tensor())
        outs = bass2jax._bass_exec_p.bind(
            *operands,
            out_avals=tuple(out_avals),
            in_names=tuple(all_in_names),
            out_names=tuple(out_names),
            lowering_input_output_aliases=(),
            sim_require_finite=True,
            sim_require_nnan=True,
            nc=nc,
        )
        return tuple(outs)

    devices = jax.devices()[:N_CORES]
    mesh = Mesh(np.asarray(devices), ("core",))
    in_specs = (PartitionSpec("core"),) * (n_params + len(out_names))
    out_specs = (PartitionSpec("core"),) * len(out_names)
    sharded = jax.jit(
        shard_map(_body, mesh=mesh, in_specs=in_specs, out_specs=out_specs,
                  check_rep=False),
        donate_argnums=donate, keep_unused=True)

    def run(in_maps):
        per_core = [[np.asarray(m[n]) for n in in_names] for m in in_maps]
        concat_in = [np.concatenate([per_core[c][i] for c in range(N_CORES)],
                                    axis=0) for i in range(n_params)]
        concat_zeros = [np.zeros((N_CORES * z.shape[0], *z.shape[1:]), z.dtype)
                        for z in zero_outs]
        out_arrs = sharded(*concat_in, *concat_zeros)
        out_arrs = [np.asarray(a) for a in out_arrs]
        return [
            {name: out_arrs[i].reshape(N_CORES, *out_avals[i].shape)[c]
             for i, name in enumerate(out_names)}
            for c in range(N_CORES)
        ]

    _CACHE[key] = run
    return run


def _make_consts(inputs):
    return {
        "w0": np.ascontiguousarray(
            np.asarray(inputs["W0"], np.float32).astype(np.float16).reshape(8, 128, 128)),
        "w1": np.ascontiguousarray(
            np.asarray(inputs["W1"], np.float32).astype(np.float16).reshape(16, 128, 128)),
        "w2": np.ascontiguousarray(
            np.asarray(inputs["W2"], np.float32).astype(np.float16).reshape(16, 128, 128)),
        "b0h": np.ascontiguousarray(
            np.asarray(inputs["b0"], np.float32)[:64].reshape(64, 1)),
        "b1h": np.ascontiguousarray(
            np.asarray(inputs["b1"], np.float32)[:64].reshape(64, 1)),
        "ident": np.eye(128, dtype=np.float32),
    }


def _prep_core_inputs(x_shard, consts):
    xt16 = np.ascontiguousarray(
        x_shard.transpose(1, 0, 2).reshape(F0, BE)).astype(np.float16)
    a0 = np.ascontiguousarray(
        np.repeat(xt16, 32, axis=0).reshape(8, 128, BE))
    a1 = np.ascontiguousarray(
        np.repeat(xt16, 64, axis=0).reshape(16, 128, BE))
    x4 = np.ascontiguousarray(np.tile(xt16, (4, 1)))
    return {"a0": a0, "a1": a1, "x4": x4, **consts}


def kernel(x, W0, b0, W1, b1, W2, b2):
    x = np.asarray(x, dtype=np.float32)
    W0 = np.asarray(W0, dtype=np.float32)
    W1 = np.asarray(W1, dtype=np.float32)
    W2 = np.asarray(W2, dtype=np.float32)
    b0 = np.asarray(b0, dtype=np.float32)
    b1 = np.asarray(b1, dtype=np.float32)
    b2 = np.asarray(b2, dtype=np.float32)

    repeat = int(os.environ.get("KERNEL_REPEAT", "1"))
    if _CACHE.get("repeat") != repeat:
        _CACHE["nc"] = _build_module(repeat)
        _CACHE["repeat"] = repeat
    nc = _CACHE["nc"]

    consts = _make_consts(
        {"W0": W0, "W1": W1, "W2": W2, "b0": b0, "b1": b1})
    in_maps = [
        _prep_core_inputs(x[i * BSH:(i + 1) * BSH], consts)
        for i in range(N_CORES)
    ]

    res = run_bass_kernel_spmd(nc, in_maps, list(range(N_CORES)))
    global last_exec_time_ns, last_results
    last_exec_time_ns = res.exec_time_ns
    last_results = res

    out = np.concatenate([np.asarray(res.results[i]["out"], dtype=np.float32)
                          for i in range(N_CORES)], axis=0)
    # direct-part bias contributions, exact in fp32 (sum over E positions)
    out[:, 0:64] += E * b0[64:128]
    out[:, 64:128] += E * b1[64:128]
    out[:, 128:256] += E * b2
    return out
